# revision 1
# baseline (speedup 1.0000x reference)
"""Trainium2 Bass kernel for nn_DistHead (block-diagonal molecule attention).

out = softmax_blockdiag(Q K^T / sqrt(H)) * exp(-invr0 * cdist(Z, Z)) @ V
with Q/K/V = X @ W{q,k,v}^T, block-diagonal over 128 molecules of 64 atoms.

Sharding: 16 whole molecules (1024 rows) per core across 8 cores —
perfectly parallel, zero cross-core communication.

Key tricks:
- Block-diagonal mask folded into the score matmul: Q^T/K^T get two extra
  contraction rows (+-16 patterns) that add exactly 0 for same-molecule
  pairs and -512 for cross-molecule pairs inside a 128-row tile, so
  exp() underflows off-block scores to exactly 0. No mask ops at all.
- Pairwise distance^2 via one K=5 matmul using augmented coordinates
  [z2, 1, -2z] x [1, z2, z].
- All elementwise work batched into single [128, 1024]-wide ops.

Self-contained: hardcodes shapes from the problem spec; only imports
concourse from /opt/trn_rl_repo.
"""

import sys

if "/opt/trn_rl_repo" not in sys.path:
    sys.path.insert(0, "/opt/trn_rl_repo")

import numpy as np

N, E, H = 8192, 256, 64          # atoms, embedding, head size
NSEG, SEG = 128, 64              # molecules, atoms per molecule
NCORES = 8
RPC = N // NCORES                # rows per core (1024 = 16 molecules)
NT = RPC // 128                  # 128-row tiles per core (2 molecules each)
EC = E // 128                    # embedding chunks of 128

_cache = {}


def _build_nc():
    import concourse.bacc as bacc
    import concourse.tile as tile
    from concourse import mybir

    f32 = mybir.dt.float32
    f16 = mybir.dt.float16
    AF = mybir.ActivationFunctionType
    X_AX = mybir.AxisListType.X

    nc = bacc.Bacc(None, target_bir_lowering=False, debug=False)

    # fp16 operands for all matmuls except the distance gram matmul (kept
    # fp32: catastrophic cancellation for nearby atoms needs the mantissa).
    xt_d = nc.dram_tensor("xt", [128, EC, RPC], f16, kind="ExternalInput")
    ident_d = nc.dram_tensor("ident", [128, 128], f16, kind="ExternalInput")
    w_d = nc.dram_tensor("w", [128, 6, H], f16, kind="ExternalInput")
    zat_d = nc.dram_tensor("zat", [5, RPC], f32, kind="ExternalInput")
    zbt_d = nc.dram_tensor("zbt", [5, RPC], f32, kind="ExternalInput")
    qaug_d = nc.dram_tensor("qaug", [2, RPC], f16, kind="ExternalInput")
    kaug_d = nc.dram_tensor("kaug", [2, RPC], f16, kind="ExternalInput")
    y_d = nc.dram_tensor("y", [RPC, H], f32, kind="ExternalOutput")

    with tile.TileContext(nc) as tc:
        with (
            tc.tile_pool(name="consts", bufs=1) as consts,
            tc.tile_pool(name="sb", bufs=4) as sb,
            tc.tile_pool(name="sm", bufs=2) as sm,
            tc.tile_pool(name="wide", bufs=1) as wide,
            tc.tile_pool(name="psm", bufs=4, space="PSUM") as psm,
            tc.tile_pool(name="psb", bufs=1, space="PSUM") as psb,
        ):
            zat = consts.tile([5, RPC], f32, tag="zat")
            nc.sync.dma_start(out=zat, in_=zat_d[:, :])
            zbt = consts.tile([5, RPC], f32, tag="zbt")
            nc.sync.dma_start(out=zbt, in_=zbt_d[:, :])

            w_sb = consts.tile([128, 6, H], f16, tag="w")
            nc.sync.dma_start(out=w_sb, in_=w_d[:, :, :])
            xt = consts.tile([128, EC, RPC], f16, tag="xt")
            for c in range(EC):
                nc.sync.dma_start(out=xt[:, c, :], in_=xt_d[:, c, :])

            # Distance^2 matmuls + decay chain first: they only need the tiny
            # zat/zbt DMAs, so the Sqrt/Exp ACT table loads and the whole
            # dist pipeline overlap the xt load and QK phase.
            d_ps = psb.tile([128, NT, 128], f32, tag="d")
            with tc.high_priority():
                for t in range(NT):
                    rt = slice(t * 128, (t + 1) * 128)
                    nc.tensor.matmul(d_ps[:, t, :], lhsT=zat[:, rt], rhs=zbt[:, rt], start=True, stop=True)
                dist = wide.tile([128, NT, 128], f32, tag="dist")
                nc.vector.tensor_scalar_max(out=dist, in0=d_ps, scalar1=0.0)
                nc.scalar.activation(out=dist, in_=dist, func=AF.Sqrt)
                disth = wide.tile([128, NT, 128], f16, tag="disth")
                nc.scalar.activation(out=disth, in_=dist, func=AF.Exp, scale=-1.0)

            ident = consts.tile([128, 128], f16, tag="ident")
            nc.scalar.dma_start(out=ident, in_=ident_d[:, :])

            # Q^T / K^T with two augmented mask rows each: [66, RPC].
            qt = consts.tile([H + 2, RPC], f16, tag="qt")
            kt = consts.tile([H + 2, RPC], f16, tag="kt")
            nc.scalar.dma_start(out=qt[H : H + 2, :], in_=qaug_d[:, :])
            nc.scalar.dma_start(out=kt[H : H + 2, :], in_=kaug_d[:, :])
            for iw, dst in ((0, qt), (1, kt)):
                for h in range(RPC // 512):
                    p = psm.tile([H, 512], f32, tag="mi")
                    for c in range(EC):
                        nc.tensor.matmul(
                            p,
                            lhsT=w_sb[:, 2 * iw + c, :],
                            rhs=xt[:, c, h * 512 : (h + 1) * 512],
                            start=(c == 0),
                            stop=(c == EC - 1),
                        )
                    nc.vector.tensor_copy(out=dst[0:H, h * 512 : (h + 1) * 512], in_=p)

            # Scores for all NT tiles; each matmul's 128-col slice stays
            # inside one psum bank.
            s_ps = psb.tile([128, NT, 128], f32, tag="s")
            for t in range(NT):
                rt = slice(t * 128, (t + 1) * 128)
                nc.tensor.matmul(s_ps[:, t, :], lhsT=qt[:, rt], rhs=kt[:, rt], start=True, stop=True)

            # exp(S): off-block entries are ~-504 -> exactly 0 (in fp16 too),
            # so dense row sums and the dense PV matmul are already correct.
            # Split into halves so tiles 0-3's transposes start while the
            # second half is still in the exp/mul chain.
            HF = NT // 2
            e = wide.tile([128, NT, 128], f16, tag="e")
            wei = wide.tile([128, NT, 128], f16, tag="wei")
            rowsum = sm.tile([128, NT], f32, tag="rowsum")
            rinv = sm.tile([128, NT], f32, tag="rinv")
            for hh in range(2):
                hs = slice(hh * HF, (hh + 1) * HF)
                nc.scalar.activation(out=e[:, hs, :], in_=s_ps[:, hs, :], func=AF.Exp)
                nc.vector.tensor_mul(out=wei[:, hs, :], in0=e[:, hs, :], in1=disth[:, hs, :])
                nc.vector.reduce_sum(out=rowsum[:, hs], in_=e[:, hs, :], axis=X_AX)
                nc.vector.reciprocal(out=rinv[:, hs], in_=rowsum[:, hs])

            # V emitted after the elementwise chain: PE runs it inside the
            # bubble while ACT/DVE work. [128, NT, H] row-major.
            v_sb = consts.tile([128, NT, H], f16, tag="v")
            for t in range(NT):
                p = psm.tile([128, H], f32, tag="mi")
                for c in range(EC):
                    nc.tensor.matmul(
                        p,
                        lhsT=xt[:, c, t * 128 : (t + 1) * 128],
                        rhs=w_sb[:, 4 + c, :],
                        start=(c == 0),
                        stop=(c == EC - 1),
                    )
                nc.vector.tensor_copy(out=v_sb[:, t, :], in_=p)

            # Transpose + PV, software-pipelined so the PE never waits on the
            # psum->sbuf hop: all transposes can run back to back.
            o_all = sb.tile([128, NT, H], f32, tag="o_all")
            wt_ps = [None] * NT
            wt_sb = [None] * NT
            for t in range(NT):
                wt_ps[t] = psm.tile([128, 128], f16, name=f"wt_ps{t}", tag="mi")
                nc.tensor.transpose(wt_ps[t], wei[:, t, :], ident)
                wt_sb[t] = sb.tile([128, 128], f16, name=f"wt_sb{t}", tag="wt")
                nc.vector.tensor_copy(out=wt_sb[t], in_=wt_ps[t])
                if t >= 1:
                    _pv(nc, psm, o_all, wt_sb, v_sb, rinv, t - 1)
            _pv(nc, psm, o_all, wt_sb, v_sb, rinv, NT - 1)
            y_r = y_d.rearrange("(t p) h -> p t h", p=128)
            nc.sync.dma_start(out=y_r[:, 0:HF, :], in_=o_all[:, 0:HF, :])
            nc.sync.dma_start(out=y_r[:, HF:NT, :], in_=o_all[:, HF:NT, :])

    nc.compile()
    return nc


def _pv(nc, psm, o_all, wt_sb, v_sb, rinv, t):
    from concourse import mybir

    f32 = mybir.dt.float32
    o_ps = psm.tile([128, H], f32, tag="mi")
    nc.tensor.matmul(o_ps, lhsT=wt_sb[t], rhs=v_sb[:, t, :], start=True, stop=True)
    nc.vector.tensor_scalar_mul(out=o_all[:, t, :], in0=o_ps, scalar1=rinv[:, t : t + 1])


def _get_nc():
    if "nc" not in _cache:
        _cache["nc"] = _build_nc()
    return _cache["nc"]


def _prepare_in_maps(X, Z, Wk, Wq, Wv, invr0):
    X = np.ascontiguousarray(X, dtype=np.float32)
    Z = np.ascontiguousarray(Z, dtype=np.float32)
    # [128, EC, N] fp16: partition p, chunk c -> X^T row c*128+p.
    xt_full = np.ascontiguousarray(
        X.T.reshape(EC, 128, N).transpose(1, 0, 2).astype(np.float16)
    )

    # invr0 folded into the distance operands: dist_psum = invr0^2 * d2,
    # so after sqrt the decay is exp(-1.0 * x).
    inv = np.float32(np.asarray(invr0).reshape(-1)[0])
    z2 = np.sum(Z * Z, axis=-1)
    ones = np.ones(N, dtype=np.float32)
    zt = np.ascontiguousarray(Z.T)
    zat_full = (inv * np.concatenate([z2[None], ones[None], -2.0 * zt], axis=0)).astype(np.float32)
    zbt_full = (inv * np.concatenate([ones[None], z2[None], zt], axis=0)).astype(np.float32)

    scale = np.float32(H) ** np.float32(-0.5)
    # [128, 6, H]: chunks [wq0, wq1, wk0, wk1, wv0, wv1].
    w_parts = [
        (Wq.T * scale).astype(np.float32).reshape(EC, 128, H),
        Wk.T.astype(np.float32).reshape(EC, 128, H),
        Wv.T.astype(np.float32).reshape(EC, 128, H),
    ]
    w_full = np.ascontiguousarray(
        np.stack([p[c] for p in w_parts for c in range(EC)], axis=1).astype(np.float16)
    )

    # Mask rows: same-molecule pairs within a 128-row tile add exactly 0,
    # cross-molecule pairs add -512 (256 and +-16 are exact in fp16).
    sig = np.where((np.arange(RPC) % 128) < SEG, 16.0, -16.0).astype(np.float16)
    ones_r = np.ones(RPC, dtype=np.float16)
    qaug = np.ascontiguousarray(np.stack([ones_r, sig]).astype(np.float16))
    kaug = np.ascontiguousarray(np.stack([-256.0 * ones_r, sig]).astype(np.float16))

    in_maps = []
    for d in range(NCORES):
        s, e = d * RPC, (d + 1) * RPC
        in_maps.append(
            {
                "xt": np.ascontiguousarray(xt_full[:, :, s:e]),
                "zat": np.ascontiguousarray(zat_full[:, s:e]),
                "zbt": np.ascontiguousarray(zbt_full[:, s:e]),
                "w": w_full,
                "ident": np.eye(128, dtype=np.float16),
                "qaug": qaug,
                "kaug": kaug,
            }
        )
    return in_maps


def _run(in_maps, trace=False, **kwargs):
    from concourse.bass_utils import run_bass_kernel_spmd

    nc = _get_nc()
    return run_bass_kernel_spmd(nc, in_maps, list(range(NCORES)), trace=trace, **kwargs)


def _numpy_fallback(X, Z, Wk, Wq, Wv, invr0, ptr):
    """Reference-exact fallback for ptr layouts other than 128 x 64."""
    X = np.asarray(X, dtype=np.float32)
    Z = np.asarray(Z, dtype=np.float32)
    n = X.shape[0]
    K = X @ Wk.T
    Q = X @ Wq.T
    V = X @ Wv.T
    seg = np.searchsorted(np.asarray(ptr)[1:], np.arange(n), side="right")
    out = np.zeros((n, Wk.shape[0]), dtype=np.float32)
    inv = float(np.asarray(invr0).reshape(-1)[0])
    hs = Wk.shape[0] ** -0.5
    for s in np.unique(seg):
        idx = np.nonzero(seg == s)[0]
        q, k, v, z = Q[idx], K[idx], V[idx], Z[idx]
        wei = (q @ k.T) * hs
        wei = wei - wei.max(axis=-1, keepdims=True)
        wei = np.exp(wei)
        wei /= wei.sum(axis=-1, keepdims=True)
        d2 = np.maximum(
            (z * z).sum(-1)[:, None] + (z * z).sum(-1)[None, :] - 2.0 * (z @ z.T), 0.0
        )
        dist = np.sqrt(np.where(d2 > 0, d2, 1.0)) * (d2 > 0)
        wei = wei * np.exp(-inv * dist)
        out[idx] = wei @ v
    return out


def kernel(X, Z, Wk, Wq, Wv, invr0, ptr):
    ptr = np.asarray(ptr)
    if not (
        X.shape == (N, E)
        and Wk.shape == (H, E)
        and ptr.shape == (NSEG + 1,)
        and np.array_equal(ptr, np.arange(NSEG + 1, dtype=ptr.dtype) * SEG)
    ):
        return _numpy_fallback(X, Z, Wk, Wq, Wv, invr0, ptr)

    in_maps = _prepare_in_maps(X, Z, Wk, Wq, Wv, invr0)
    res = _run(in_maps, trace=False)
    out = np.empty((N, H), dtype=np.float32)
    for d in range(NCORES):
        out[d * RPC : (d + 1) * RPC] = res.results[d]["y"]
    return out

